# revision 38
# baseline (speedup 1.0000x reference)
"""Trainium2 Bass kernel for nn_Actor (gnn_message_passing).

Per-batch math (B=8, N=1024, D=128), one batch element per NeuronCore:
  k_mu,v_mu,k_sg,v_sg = split(kv)                  [N,D] each
  S1 = k_mu @ v_mu^T ; S2 = k_sg @ v_sg^T          [N,N]
  P[i,j,:] = (pos_i - pos_j)/(|pos_i - pos_j|+1e-8)
  mean  = sum_j P * S1 ; lstd = clip(sum_j P * S2, -20, 2)
  tanh-Normal rsample epilogue -> actions [N,3], log_prob [1]

Factorization: with W = S/|d_ij| (diag zeroed),
  mean[i,c] = pos[i,c]*rowsum(W)[i] - (W @ pos)[i,c]
so no [N,N,3] tensor is ever formed.  The device kernel computes W^T
tiles [j_blk=128, i=1024] so the j-contraction is a plain matmul with
stationary P4 = [pos,1].

d^2 comes from ONE 18-dim bf16 bilinear matmul: positions and -r/2 are
split hi/lo(/lo2) into bf16 so every partial product is exact in the
fp32 PSUM accumulation; the result is |p_hat_i - p_hat_j|^2 up to
~3e-5, while the min off-diag d^2 is 1.76e-4 -> strictly positive.
The diagonal is pushed to ~1e14 by accumulating I*1e7 x I*(-5e6), so
its W contribution is ~1e-7 (reference has exactly 0).
invn = Abs_reciprocal_sqrt(-2*H) in a single ScalarE pass; tanh is
computed from exp via tanh(x) = sign(x)*(1-e^{-2|x|})/(1+e^{-2|x|}).
Activation->table-set bindings are pinned so the kernel performs
exactly two ACT_TABLE_LOADs.  kv arrives host-transposed (kvT) so the
[d,N] matmul operands need no on-device PE transposes, only cheap
f32r rounding copies.
"""

import os
import sys

for _p in ("/opt/trn_rl_repo", "/root/.axon_site/_ro/trn_rl_repo"):
    if os.path.isdir(_p) and _p not in sys.path:
        sys.path.append(_p)

from contextlib import ExitStack

import numpy as np
import ml_dtypes

import concourse.bass as bass
import concourse.bacc as bacc
import concourse.tile as tile
import concourse.mybir as mybir
from concourse.bass_utils import run_bass_kernel_spmd

F32 = mybir.dt.float32
F32R = mybir.dt.float32r
BF16 = mybir.dt.bfloat16
AF = mybir.ActivationFunctionType
ALU = mybir.AluOpType

B, N, D = 8, 1024, 128
NB = N // 128  # 8 row blocks
LOG_STD_MIN, LOG_STD_MAX = -20.0, 2.0
ACTION_SCALE = 0.01
C_NLP = float(0.5 * np.log(2.0 * np.pi))   # 0.9189385
LOG2x2 = float(2.0 * np.log(2.0))          # 1.3862944
LP_BIAS = float(-(N * 3) * np.log(ACTION_SCALE))


def _patch_act_tables():
    """Pin activation->table-set binding so the kernel does exactly two
    ACT_TABLE_LOADs: Copy + Abs_reciprocal_sqrt (main loop, evacuations)
    live in abs_reciprocal_sqrt_and_small; Exp/Ln/Abs/Sign (epilogue)
    live in natural_log_exp_and_others."""
    if getattr(bacc, "_ant_act_tables_patched", False):
        return
    epi = {AF.Exp, AF.Ln, AF.Abs, AF.Sign, AF.Square, AF.Identity}
    ars = "abs_reciprocal_sqrt_and_small"
    nle = "natural_log_exp_and_others"
    orig = bacc.get_activation_tables

    def patched(arch):
        t = orig(arch)
        out = {}
        for name, fns in t.items():
            fns = set(fns)
            if name == ars:
                fns -= epi
            elif name == nle:
                fns -= {AF.Copy}
            else:
                fns -= epi | {AF.Copy}
            out[name] = fns
        return out

    bacc.get_activation_tables = patched
    bacc._ant_act_tables_patched = True


def build_nc():
    _patch_act_tables()
    nc = bacc.Bacc(None, target_bir_lowering=False)

    kvt_d = nc.declare_dram_parameter("kvt", [4 * D, N], F32,
                                      isOutput=False)
    pos_d = nc.declare_dram_parameter("pos", [N, 3], F32, isOutput=False)
    eps_d = nc.declare_dram_parameter("eps", [N, 3], F32, isOutput=False)
    eyef_d = nc.declare_dram_parameter("eyef", [128, 128], F32,
                                       isOutput=False)
    eyeba_d = nc.declare_dram_parameter("eyeba", [128, 128], BF16,
                                        isOutput=False)  # I*1e7
    eyebb_d = nc.declare_dram_parameter("eyebb", [128, 128], BF16,
                                        isOutput=False)  # I*-5e6
    ones_d = nc.declare_dram_parameter("onescol", [128, 1], F32,
                                       isOutput=False)
    ones3b_d = nc.declare_dram_parameter("ones3b", [3, N], BF16,
                                         isOutput=False)
    hl_d = nc.declare_dram_parameter("hlrows", [6, N], BF16, isOutput=False)
    abc_d = nc.declare_dram_parameter("abcrows", [3, N], BF16,
                                      isOutput=False)
    pospk_d = nc.declare_dram_parameter("pospk", [128, 24], F32,
                                        isOutput=False)
    epspk_d = nc.declare_dram_parameter("epspk", [128, 24], F32,
                                        isOutput=False)
    act_d = nc.declare_dram_parameter("actions", [128, 24], F32,
                                      isOutput=True)
    lp_d = nc.declare_dram_parameter("logprob", [1, 1], F32, isOutput=True)

    with tile.TileContext(nc) as tc, ExitStack() as ctx:
        cpool = ctx.enter_context(tc.tile_pool(name="consts", bufs=1))
        kvp = ctx.enter_context(tc.tile_pool(name="kvp", bufs=3))
        tmat = ctx.enter_context(tc.tile_pool(name="tmat", bufs=1))
        small = ctx.enter_context(tc.tile_pool(name="small", bufs=1))
        p4p = ctx.enter_context(tc.tile_pool(name="p4p", bufs=8))
        invp = ctx.enter_context(tc.tile_pool(name="invp", bufs=4))
        wp = ctx.enter_context(tc.tile_pool(name="wp", bufs=4))

        # ---- constants ----
        eyef = cpool.tile([128, 128], F32, tag="eyef")
        nc.gpsimd.dma_start(eyef[:], eyef_d[:, :])
        eyeba = cpool.tile([128, 128], BF16, tag="eyeba")
        nc.gpsimd.dma_start(eyeba[:], eyeba_d[:, :])
        eyebb = cpool.tile([128, 128], BF16, tag="eyebb")
        nc.gpsimd.dma_start(eyebb[:], eyebb_d[:, :])
        onescol = cpool.tile([128, 1], F32, tag="onescol")
        nc.gpsimd.dma_start(onescol[:], ones_d[:, :])

        # ---- positions: natural tiles (P4) + packed tiles (epilogue) ----
        pos_nat = []
        for ib in range(NB):
            pn = cpool.tile([128, 3], F32, tag=f"posn{ib}")
            nc.sync.dma_start(pn[:], pos_d[ib * 128:(ib + 1) * 128, :])
            pos_nat.append(pn)

        # packed [128, 24]: col = ic*3+c, row p -> i = ic*128+p
        pos_pk = small.tile([128, 24], F32, tag="pos_pk")
        eps_pk = small.tile([128, 24], F32, tag="eps_pk")
        nc.sync.dma_start(
            pos_pk[:].rearrange("p (ic c) -> p ic c", c=3),
            pos_d.rearrange("(ic p) c -> p ic c", p=128))
        nc.sync.dma_start(
            eps_pk[:].rearrange("p (ic c) -> p ic c", c=3),
            eps_d.rearrange("(ic p) c -> p ic c", p=128))

        # P4[jb] = [pos, 1, 0...]: stationary lhsT of the accumulation
        # matmuls, padded to M=32 so the lstd copies can col-tile to the
        # 32:64 partition group of the same PSUM bank.
        p4s = []
        for jb in range(NB):
            p4 = p4p.tile([128, 4], F32R, tag="p4")
            nc.vector.tensor_copy(p4[:, 0:3], pos_nat[jb][:])
            nc.vector.tensor_copy(p4[:, 3:4], onescol[:])
            p4s.append(p4)

        # ---- 18-dim bf16 bilinear form for d^2 ----
        # H[j,i] = p^_j . p^_i - r^_i/2 - r^_j/2 with p^ = h + l (bf16
        # split of pos) and -r^/2 = ra+rb+rc (3-term bf16 split); d2=-2H.
        # All rows are host-derived O(N) marshalling, loaded contiguously.
        # A18 rows: [h, l, h, l, ra rb rc, 1 1 1]
        # B18 rows: [h, h, l, l, 1 1 1, ra rb rc]
        A18 = small.tile([18, N], BF16, tag="A18")
        B18 = small.tile([18, N], BF16, tag="B18")
        nc.scalar.dma_start(A18[0:6, :], hl_d[:, :])
        nc.scalar.dma_start(A18[6:12, :], hl_d[:, :])
        nc.scalar.dma_start(B18[0:3, :], hl_d[0:3, :])
        nc.scalar.dma_start(B18[3:6, :], hl_d[0:3, :])
        nc.scalar.dma_start(B18[6:9, :], hl_d[3:6, :])
        nc.scalar.dma_start(B18[9:12, :], hl_d[3:6, :])
        nc.scalar.dma_start(A18[12:15, :], abc_d[:, :])
        nc.scalar.dma_start(B18[15:18, :], abc_d[:, :])
        nc.scalar.dma_start(A18[15:18, :], ones3b_d[:, :])
        nc.scalar.dma_start(B18[12:15, :], ones3b_d[:, :])

        # ---- kv^T (host-transposed) load + f32r rounding copies ----
        t_mu = tmat.tile([128, N], F32R, tag="t_mu")
        u_mu = tmat.tile([128, N], F32R, tag="u_mu")
        t_sg = tmat.tile([128, N], F32R, tag="t_sg")
        u_sg = tmat.tile([128, N], F32R, tag="u_sg")
        dests = [t_mu, u_mu, t_sg, u_sg]
        stg = ctx.enter_context(tc.tile_pool(name="stg", bufs=8))
        for m in range(4):
            for hh in range(2):
                st = stg.tile([128, 512], F32, tag="st", name=f"st{m}{hh}")
                eng = nc.sync if (2 * m + hh) % 2 == 0 else nc.scalar
                eng.dma_start(st[:],
                              kvt_d[m * 128:(m + 1) * 128,
                                    hh * 512:(hh + 1) * 512])
                dst = dests[m][:, hh * 512:(hh + 1) * 512]
                if m < 1:
                    nc.scalar.copy(dst, st[:])
                else:
                    nc.vector.tensor_copy(dst, st[:])

        # ---- PSUM pools for the main loop (6 mm banks + 2 acc) ----
        mmp = ctx.enter_context(tc.tile_pool(name="mmp", bufs=6, space="PSUM"))
        accp = ctx.enter_context(tc.tile_pool(name="accp", bufs=1,
                                              space="PSUM"))
        evM = [small.tile([4, 512], F32, tag=f"evM{ih}", name=f"evM{ih}")
               for ih in range(2)]
        evL = [small.tile([4, 512], F32, tag=f"evL{ih}", name=f"evL{ih}")
               for ih in range(2)]

        for ih in range(2):
            i0 = ih * 512
            accM = accp.tile([4, 512], F32, tag="accM", name=f"accM{ih}")
            accL = accp.tile([4, 512], F32, tag="accL", name=f"accL{ih}")
            for jb in range(NB):
                j0 = jb * 128
                diag_here = (j0 >= i0) and (j0 < i0 + 512)
                c0 = j0 - i0

                h2 = mmp.tile([128, 512], F32, tag="mm", name="h2")
                nc.tensor.matmul(h2[:], B18[:, j0:j0 + 128],
                                 A18[:, i0:i0 + 512],
                                 start=True, stop=not diag_here)
                if diag_here:
                    nc.tensor.matmul(h2[:, c0:c0 + 128], eyeba[:], eyebb[:],
                                     start=False, stop=True)

                invn = invp.tile([128, 512], F32, tag="invn")
                nc.scalar.activation(invn[:], h2[:],
                                     AF.Abs_reciprocal_sqrt, scale=-2.0)

                s1 = mmp.tile([128, 512], F32, tag="mm", name="s1")
                nc.tensor.matmul(s1[:], u_mu[:, j0:j0 + 128],
                                 t_mu[:, i0:i0 + 512], start=True, stop=True)
                w1 = wp.tile([128, 512], F32R, tag="w1")
                nc.vector.tensor_mul(w1[:], s1[:], invn[:])

                s2 = mmp.tile([128, 512], F32, tag="mm", name="s2")
                nc.tensor.matmul(s2[:], u_sg[:, j0:j0 + 128],
                                 t_sg[:, i0:i0 + 512], start=True, stop=True)
                w2 = wp.tile([128, 512], F32R, tag="w2")
                nc.vector.tensor_mul(w2[:], s2[:], invn[:])

                first, last = (jb == 0), (jb == NB - 1)
                lhs4 = p4s[jb][:, :]
                nc.tensor.matmul(accM[:], lhs4, w1[:],
                                 start=first, stop=last)
                nc.tensor.matmul(accL[:], lhs4, w2[:],
                                 start=first, stop=last)
            # evacuate, freeing the two acc banks for the next half
            nc.scalar.copy(evM[ih][:], accM[:])
            nc.scalar.copy(evL[ih][:], accL[:])

        # ---- epilogue ----
        # transpose [4,128] chunks into one packed psum bank [128, 64]:
        # col ic*4+c = (W1@pos | R1), col 32+ic*4+c = (W2@pos | R2)
        pk = mmp.tile([128, 64], F32, tag="mm")
        tcount = 0
        for ic in range(NB):
            cc = (ic % 4) * 128
            for base, src in ((0, evM[ic // 4]), (32, evL[ic // 4])):
                nc.tensor.matmul(pk[:, base + ic * 4: base + ic * 4 + 4],
                                 src[0:4, cc:cc + 128],
                                 eyef[0:4, 0:4],
                                 is_transpose=True,
                                 start=(tcount == 0), stop=(tcount == 15))
                tcount += 1
        E = small.tile([128, 64], F32, tag="E")
        nc.vector.tensor_copy(E[:], pk[:])

        def e4(base):  # [128, 8, 4] view of E half
            return E[:, base:base + 32].rearrange("p (ic f) -> p ic f", f=4)

        wm = e4(0)[:, :, 0:3]    # (W1@pos)^T packed
        wl = e4(32)[:, :, 0:3]   # (W2@pos)^T packed

        # replicate R (col 3 of each 4-group) across the 3 channels
        r3m = small.tile([128, 24], F32, tag="r3m")
        r3l = small.tile([128, 24], F32, tag="r3l")
        for c in range(3):
            nc.vector.tensor_copy(
                r3m[:].rearrange("p (ic c) -> p ic c", c=3)[:, :, c:c + 1],
                e4(0)[:, :, 3:4])
            nc.vector.tensor_copy(
                r3l[:].rearrange("p (ic c) -> p ic c", c=3)[:, :, c:c + 1],
                e4(32)[:, :, 3:4])

        def t24(tag):
            return small.tile([128, 24], F32, tag=tag, name=tag)

        mean = t24("mean")
        mean_v = mean[:].rearrange("p (ic c) -> p ic c", c=3)
        nc.vector.tensor_mul(mean[:], pos_pk[:], r3m[:])
        nc.vector.tensor_sub(mean_v, mean_v, wm)

        lstd = t24("lstd")
        lstd_v = lstd[:].rearrange("p (ic c) -> p ic c", c=3)
        nc.vector.tensor_mul(lstd[:], pos_pk[:], r3l[:])
        nc.vector.tensor_sub(lstd_v, lstd_v, wl)
        nc.vector.tensor_scalar(lstd[:], lstd[:], LOG_STD_MIN, LOG_STD_MAX,
                                ALU.max, ALU.min)
        # with accum_out, op1 is the reduce op (add): sums clipped lstd
        s_lstd = small.tile([128, 1], F32, tag="s_lstd")
        lsum = t24("lsum")
        nc.vector.tensor_scalar(lsum[:], lstd[:], 0.0, None,
                                ALU.add, ALU.add, accum_out=s_lstd[:])

        std = t24("std")
        nc.scalar.activation(std[:], lstd[:], AF.Exp)
        pre = t24("pre")
        nc.vector.tensor_mul(pre[:], std[:], eps_pk[:])
        nc.vector.tensor_add(pre[:], mean[:], pre[:])

        # |pre| and u = exp(-2|pre|) (shared by tanh and log-det)
        ap_t = t24("ap_t")
        s_ap = small.tile([128, 1], F32, tag="s_ap")
        nc.scalar.activation(ap_t[:], pre[:], AF.Abs)
        apsum = t24("apsum")
        nc.vector.tensor_scalar(apsum[:], ap_t[:], 0.0, None,
                                ALU.add, ALU.add, accum_out=s_ap[:])
        u_t = t24("u_t")
        nc.scalar.activation(u_t[:], ap_t[:], AF.Exp, scale=-2.0)

        # actions = sign(pre) * (1-u)/(1+u) * ACTION_SCALE
        sgn = t24("sgn")
        nc.scalar.activation(sgn[:], pre[:], AF.Sign)
        den = t24("den")
        nc.vector.tensor_scalar_add(den[:], u_t[:], 1.0)
        rec = t24("rec")
        nc.vector.reciprocal(rec[:], den[:])
        num = t24("num")
        nc.vector.tensor_scalar(num[:], u_t[:], -1.0, 1.0, ALU.mult, ALU.add)
        acts = t24("acts")
        nc.vector.tensor_mul(acts[:], num[:], rec[:])
        nc.vector.tensor_mul(acts[:], acts[:], sgn[:])
        nc.vector.tensor_scalar_mul(acts[:], acts[:], ACTION_SCALE)
        nc.sync.dma_start(act_d[:, :], acts[:])

        # log_prob pieces
        delta = t24("delta")
        nc.vector.tensor_sub(delta[:], pre[:], mean[:])
        istd = t24("istd")
        nc.scalar.activation(istd[:], lstd[:], AF.Exp, scale=-1.0)
        td = t24("td")
        nc.vector.tensor_mul(td[:], delta[:], istd[:])
        sqq = t24("sqq")
        s_sq = small.tile([128, 1], F32, tag="s_sq")
        nc.vector.scalar_tensor_tensor(sqq[:], td[:], 0.0, td[:],
                                       ALU.add, ALU.mult, accum_out=s_sq[:])

        v_t = t24("v_t")
        s_v = small.tile([128, 1], F32, tag="s_v")
        nc.scalar.activation(v_t[:], u_t[:], AF.Ln, bias=1.0)
        vsum = t24("vsum")
        nc.vector.tensor_scalar(vsum[:], v_t[:], 0.0, None,
                                ALU.add, ALU.add, accum_out=s_v[:])

        # L = -0.5*s_sq - s_lstd + 2*s_ap + 2*s_v - 24*(C_NLP + LOG2x2)
        L = small.tile([128, 1], F32, tag="L")
        nc.vector.tensor_scalar_mul(L[:], s_sq[:], -0.5)
        nc.vector.scalar_tensor_tensor(L[:], s_ap[:], 2.0, L[:],
                                       ALU.mult, ALU.add)
        nc.vector.scalar_tensor_tensor(L[:], s_v[:], 2.0, L[:],
                                       ALU.mult, ALU.add)
        nc.vector.scalar_tensor_tensor(L[:], s_lstd[:], -1.0, L[:],
                                       ALU.mult, ALU.add)
        nc.vector.tensor_scalar_add(L[:], L[:], -24.0 * (C_NLP + LOG2x2))

        lp_ps = mmp.tile([1, 1], F32, tag="mm")
        nc.tensor.matmul(lp_ps[:], onescol[:], L[:], start=True, stop=True)
        lp_sb = small.tile([1, 1], F32, tag="lp_sb")
        nc.vector.tensor_scalar_add(lp_sb[:], lp_ps[:], LP_BIAS)
        nc.sync.dma_start(lp_d[:, :], lp_sb[:])

    nc.finalize()
    return nc


_NC_CACHE = None


def _get_nc():
    global _NC_CACHE
    if _NC_CACHE is None:
        _NC_CACHE = build_nc()
    return _NC_CACHE


def make_in_maps(kv, positions, eps):
    bf16 = ml_dtypes.bfloat16
    eyef = np.eye(128, dtype=np.float32)
    eyeba = (np.eye(128) * 1e7).astype(bf16)
    eyebb = (np.eye(128) * -5e6).astype(bf16)
    onescol = np.ones((128, 1), np.float32)
    ones3b = np.ones((3, N), bf16)
    maps = []
    for i in range(B):
        pos = np.ascontiguousarray(positions[i], np.float32)
        epsb = np.ascontiguousarray(eps[i], np.float32)
        h = pos.astype(bf16)
        l = (pos - h.astype(np.float32)).astype(bf16)
        ph = (h.astype(np.float32) + l.astype(np.float32)).astype(np.float32)
        sq = (ph * ph).astype(np.float32)
        r = (sq[:, 0] + sq[:, 1]).astype(np.float32)
        r = (r + sq[:, 2]).astype(np.float32)
        x = (-0.5 * r).astype(np.float32)
        ra = x.astype(bf16)
        x1 = (x - ra.astype(np.float32)).astype(np.float32)
        rb = x1.astype(bf16)
        x2 = (x1 - rb.astype(np.float32)).astype(np.float32)
        rc = x2.astype(bf16)
        hl = np.ascontiguousarray(np.concatenate([h.T, l.T], 0))
        abc = np.ascontiguousarray(np.stack([ra, rb, rc], 0))
        pos_pk = np.ascontiguousarray(
            pos.reshape(8, 128, 3).transpose(1, 0, 2).reshape(128, 24))
        eps_pk = np.ascontiguousarray(
            epsb.reshape(8, 128, 3).transpose(1, 0, 2).reshape(128, 24))
        maps.append({
            "kvt": np.ascontiguousarray(kv[i].T),
            "pos": pos,
            "eps": epsb,
            "eyef": eyef,
            "eyeba": eyeba,
            "eyebb": eyebb,
            "onescol": onescol,
            "ones3b": ones3b,
            "hlrows": hl,
            "abcrows": abc,
            "pospk": pos_pk,
            "epspk": eps_pk,
        })
    return maps


def kernel(kv, positions, eps):
    kv = np.asarray(kv, np.float32)
    positions = np.asarray(positions, np.float32)
    eps = np.asarray(eps, np.float32)
    nc = _get_nc()
    maps = make_in_maps(kv, positions, eps)
    res = run_bass_kernel_spmd(nc, maps, core_ids=list(range(B)))
    outs = res.results
    actions = np.stack([
        outs[i]["actions"].reshape(128, 8, 3).transpose(1, 0, 2)
        .reshape(N, 3) for i in range(B)])
    log_prob = np.stack([outs[i]["logprob"].reshape(1) for i in range(B)])
    return actions.astype(np.float32), log_prob.astype(np.float32)


# revision 39
# speedup vs baseline: 1.1145x; 1.1145x over previous
"""Trainium2 Bass kernel for nn_Actor (gnn_message_passing).

Per-batch math (B=8, N=1024, D=128), one batch element per NeuronCore:
  k_mu,v_mu,k_sg,v_sg = split(kv)                  [N,D] each
  S1 = k_mu @ v_mu^T ; S2 = k_sg @ v_sg^T          [N,N]
  P[i,j,:] = (pos_i - pos_j)/(|pos_i - pos_j|+1e-8)
  mean  = sum_j P * S1 ; lstd = clip(sum_j P * S2, -20, 2)
  tanh-Normal rsample epilogue -> actions [N,3], log_prob [1]

Factorization: with W = S/|d_ij| (diag zeroed),
  mean[i,c] = pos[i,c]*rowsum(W)[i] - (W @ pos)[i,c]
so no [N,N,3] tensor is ever formed.  The device kernel computes W^T
tiles [j_blk=128, i=1024] so the j-contraction is a plain matmul with
stationary P4 = [pos,1].

d^2 comes from ONE 18-dim bf16 bilinear matmul: positions and -r/2 are
split hi/lo(/lo2) into bf16 so every partial product is exact in the
fp32 PSUM accumulation; the result is |p_hat_i - p_hat_j|^2 up to
~3e-5, while the min off-diag d^2 is 1.76e-4 -> strictly positive.
The diagonal is pushed to ~1e14 by accumulating I*1e7 x I*(-5e6), so
its W contribution is ~1e-7 (reference has exactly 0).
invn = Abs_reciprocal_sqrt(-2*H) in a single ScalarE pass; tanh is
computed from exp via tanh(x) = sign(x)*(1-e^{-2|x|})/(1+e^{-2|x|}).
Activation->table-set bindings are pinned so the kernel performs
exactly two ACT_TABLE_LOADs.  kv arrives host-transposed (kvT) so the
[d,N] matmul operands need no on-device PE transposes, only cheap
f32r rounding copies.
"""

import os
import sys

for _p in ("/opt/trn_rl_repo", "/root/.axon_site/_ro/trn_rl_repo"):
    if os.path.isdir(_p) and _p not in sys.path:
        sys.path.append(_p)

from contextlib import ExitStack

import numpy as np
import ml_dtypes

import concourse.bass as bass
import concourse.bacc as bacc
import concourse.tile as tile
import concourse.mybir as mybir
from concourse.bass_utils import run_bass_kernel_spmd

F32 = mybir.dt.float32
F32R = mybir.dt.float32r
BF16 = mybir.dt.bfloat16
AF = mybir.ActivationFunctionType
ALU = mybir.AluOpType

B, N, D = 8, 1024, 128
NB = N // 128  # 8 row blocks
LOG_STD_MIN, LOG_STD_MAX = -20.0, 2.0
ACTION_SCALE = 0.01
C_NLP = float(0.5 * np.log(2.0 * np.pi))   # 0.9189385
LOG2x2 = float(2.0 * np.log(2.0))          # 1.3862944
LP_BIAS = float(-(N * 3) * np.log(ACTION_SCALE))


def _patch_act_tables():
    """Pin activation->table-set binding so the kernel does exactly two
    ACT_TABLE_LOADs: Copy + Abs_reciprocal_sqrt (main loop, evacuations)
    live in abs_reciprocal_sqrt_and_small; Exp/Ln/Abs/Sign (epilogue)
    live in natural_log_exp_and_others."""
    if getattr(bacc, "_ant_act_tables_patched", False):
        return
    epi = {AF.Exp, AF.Ln, AF.Abs, AF.Sign, AF.Square, AF.Identity}
    ars = "abs_reciprocal_sqrt_and_small"
    nle = "natural_log_exp_and_others"
    orig = bacc.get_activation_tables

    def patched(arch):
        t = orig(arch)
        out = {}
        for name, fns in t.items():
            fns = set(fns)
            if name == ars:
                fns -= epi
            elif name == nle:
                fns -= {AF.Copy}
            else:
                fns -= epi | {AF.Copy}
            out[name] = fns
        return out

    bacc.get_activation_tables = patched
    bacc._ant_act_tables_patched = True


def build_nc():
    _patch_act_tables()
    nc = bacc.Bacc(None, target_bir_lowering=False)

    kvt_d = nc.declare_dram_parameter("kvt", [4 * D, N], F32,
                                      isOutput=False)
    pos_d = nc.declare_dram_parameter("pos", [N, 3], F32, isOutput=False)
    eps_d = nc.declare_dram_parameter("eps", [N, 3], F32, isOutput=False)
    eyef_d = nc.declare_dram_parameter("eyef", [128, 128], F32,
                                       isOutput=False)
    eyeba_d = nc.declare_dram_parameter("eyeba", [128, 128], BF16,
                                        isOutput=False)  # I*1e7
    eyebb_d = nc.declare_dram_parameter("eyebb", [128, 128], BF16,
                                        isOutput=False)  # I*-5e6
    ones_d = nc.declare_dram_parameter("onescol", [128, 1], F32,
                                       isOutput=False)
    ones3b_d = nc.declare_dram_parameter("ones3b", [3, N], BF16,
                                         isOutput=False)
    hl_d = nc.declare_dram_parameter("hlrows", [6, N], BF16, isOutput=False)
    abc_d = nc.declare_dram_parameter("abcrows", [3, N], BF16,
                                      isOutput=False)
    pospk_d = nc.declare_dram_parameter("pospk", [128, 24], F32,
                                        isOutput=False)
    epspk_d = nc.declare_dram_parameter("epspk", [128, 24], F32,
                                        isOutput=False)
    act_d = nc.declare_dram_parameter("actions", [128, 24], F32,
                                      isOutput=True)
    lp_d = nc.declare_dram_parameter("logprob", [1, 1], F32, isOutput=True)

    with tile.TileContext(nc) as tc, ExitStack() as ctx:
        cpool = ctx.enter_context(tc.tile_pool(name="consts", bufs=1))
        kvp = ctx.enter_context(tc.tile_pool(name="kvp", bufs=3))
        tmat = ctx.enter_context(tc.tile_pool(name="tmat", bufs=1))
        small = ctx.enter_context(tc.tile_pool(name="small", bufs=1))
        p4p = ctx.enter_context(tc.tile_pool(name="p4p", bufs=8))
        invp = ctx.enter_context(tc.tile_pool(name="invp", bufs=4))
        wp = ctx.enter_context(tc.tile_pool(name="wp", bufs=4))

        # ---- constants ----
        eyef = cpool.tile([128, 128], F32, tag="eyef")
        nc.gpsimd.dma_start(eyef[:], eyef_d[:, :])
        eyeba = cpool.tile([128, 128], BF16, tag="eyeba")
        nc.gpsimd.dma_start(eyeba[:], eyeba_d[:, :])
        eyebb = cpool.tile([128, 128], BF16, tag="eyebb")
        nc.gpsimd.dma_start(eyebb[:], eyebb_d[:, :])
        onescol = cpool.tile([128, 1], F32, tag="onescol")
        nc.gpsimd.dma_start(onescol[:], ones_d[:, :])

        # packed [128, 24]: col = ic*3+c, row p -> i = ic*128+p
        # (host-packed, contiguous loads)
        pos_pk = small.tile([128, 24], F32, tag="pos_pk")
        eps_pk = small.tile([128, 24], F32, tag="eps_pk")
        nc.gpsimd.dma_start(pos_pk[:], pospk_d[:, :])
        nc.gpsimd.dma_start(eps_pk[:], epspk_d[:, :])

        # P4[jb] = [pos, 1, 0...]: stationary lhsT of the accumulation
        # matmuls, padded to M=32 so the lstd copies can col-tile to the
        # 32:64 partition group of the same PSUM bank.
        p4s = []
        for jb in range(NB):
            p4 = p4p.tile([128, 4], F32R, tag="p4")
            nc.vector.tensor_copy(p4[:, 0:3], pos_pk[:, jb * 3:jb * 3 + 3])
            nc.vector.tensor_copy(p4[:, 3:4], onescol[:])
            p4s.append(p4)

        # ---- 18-dim bf16 bilinear form for d^2 ----
        # H[j,i] = p^_j . p^_i - r^_i/2 - r^_j/2 with p^ = h + l (bf16
        # split of pos) and -r^/2 = ra+rb+rc (3-term bf16 split); d2=-2H.
        # All rows are host-derived O(N) marshalling, loaded contiguously.
        # A18 rows: [h, l, h, l, ra rb rc, 1 1 1]
        # B18 rows: [h, h, l, l, 1 1 1, ra rb rc]
        A18 = small.tile([18, N], BF16, tag="A18")
        B18 = small.tile([18, N], BF16, tag="B18")
        nc.scalar.dma_start(A18[0:6, :], hl_d[:, :])
        nc.scalar.dma_start(A18[6:12, :], hl_d[:, :])
        nc.scalar.dma_start(B18[0:3, :], hl_d[0:3, :])
        nc.scalar.dma_start(B18[3:6, :], hl_d[0:3, :])
        nc.scalar.dma_start(B18[6:9, :], hl_d[3:6, :])
        nc.scalar.dma_start(B18[9:12, :], hl_d[3:6, :])
        nc.scalar.dma_start(A18[12:15, :], abc_d[:, :])
        nc.scalar.dma_start(B18[15:18, :], abc_d[:, :])
        nc.scalar.dma_start(A18[15:18, :], ones3b_d[:, :])
        nc.scalar.dma_start(B18[12:15, :], ones3b_d[:, :])

        # ---- kv^T (host-transposed) load + f32r rounding copies ----
        t_mu = tmat.tile([128, N], F32R, tag="t_mu")
        u_mu = tmat.tile([128, N], F32R, tag="u_mu")
        t_sg = tmat.tile([128, N], F32R, tag="t_sg")
        u_sg = tmat.tile([128, N], F32R, tag="u_sg")
        dests = [t_mu, u_mu, t_sg, u_sg]
        stg = ctx.enter_context(tc.tile_pool(name="stg", bufs=8))
        for m in range(4):
            for hh in range(2):
                st = stg.tile([128, 512], F32, tag="st", name=f"st{m}{hh}")
                eng = nc.sync if (2 * m + hh) % 2 == 0 else nc.scalar
                eng.dma_start(st[:],
                              kvt_d[m * 128:(m + 1) * 128,
                                    hh * 512:(hh + 1) * 512])
                dst = dests[m][:, hh * 512:(hh + 1) * 512]
                nc.vector.tensor_copy(dst, st[:])

        # ---- PSUM pools for the main loop (6 mm banks + 2 acc) ----
        mmp = ctx.enter_context(tc.tile_pool(name="mmp", bufs=6, space="PSUM"))
        accp = ctx.enter_context(tc.tile_pool(name="accp", bufs=1,
                                              space="PSUM"))
        evM = [small.tile([4, 512], F32, tag=f"evM{ih}", name=f"evM{ih}")
               for ih in range(2)]
        evL = [small.tile([4, 512], F32, tag=f"evL{ih}", name=f"evL{ih}")
               for ih in range(2)]

        for ih in range(2):
            i0 = ih * 512
            accM = accp.tile([4, 512], F32, tag="accM", name=f"accM{ih}")
            accL = accp.tile([4, 512], F32, tag="accL", name=f"accL{ih}")
            for jb in range(NB):
                j0 = jb * 128
                diag_here = (j0 >= i0) and (j0 < i0 + 512)
                c0 = j0 - i0

                h2 = mmp.tile([128, 512], F32, tag="mm", name="h2")
                nc.tensor.matmul(h2[:], B18[:, j0:j0 + 128],
                                 A18[:, i0:i0 + 512],
                                 start=True, stop=not diag_here)
                if diag_here:
                    nc.tensor.matmul(h2[:, c0:c0 + 128], eyeba[:], eyebb[:],
                                     start=False, stop=True)

                invn = invp.tile([128, 512], F32, tag="invn")
                nc.scalar.activation(invn[:], h2[:],
                                     AF.Abs_reciprocal_sqrt, scale=-2.0)

                s1 = mmp.tile([128, 512], F32, tag="mm", name="s1")
                nc.tensor.matmul(s1[:], u_mu[:, j0:j0 + 128],
                                 t_mu[:, i0:i0 + 512], start=True, stop=True)
                w1 = wp.tile([128, 512], F32R, tag="w1")
                nc.vector.tensor_mul(w1[:], s1[:], invn[:])

                s2 = mmp.tile([128, 512], F32, tag="mm", name="s2")
                nc.tensor.matmul(s2[:], u_sg[:, j0:j0 + 128],
                                 t_sg[:, i0:i0 + 512], start=True, stop=True)
                w2 = wp.tile([128, 512], F32R, tag="w2")
                nc.vector.tensor_mul(w2[:], s2[:], invn[:])

                first, last = (jb == 0), (jb == NB - 1)
                lhs4 = p4s[jb][:, :]
                nc.tensor.matmul(accM[:], lhs4, w1[:],
                                 start=first, stop=last)
                nc.tensor.matmul(accL[:], lhs4, w2[:],
                                 start=first, stop=last)
            # evacuate, freeing the two acc banks for the next half
            nc.scalar.copy(evM[ih][:], accM[:])
            nc.scalar.copy(evL[ih][:], accL[:])

        # ---- epilogue ----
        # transpose [4,128] chunks into one packed psum bank [128, 64]:
        # col ic*4+c = (W1@pos | R1), col 32+ic*4+c = (W2@pos | R2)
        pk = mmp.tile([128, 64], F32, tag="mm")
        tcount = 0
        for ic in range(NB):
            cc = (ic % 4) * 128
            for base, src in ((0, evM[ic // 4]), (32, evL[ic // 4])):
                nc.tensor.matmul(pk[:, base + ic * 4: base + ic * 4 + 4],
                                 src[0:4, cc:cc + 128],
                                 eyef[0:4, 0:4],
                                 is_transpose=True,
                                 start=(tcount == 0), stop=(tcount == 15))
                tcount += 1
        E = small.tile([128, 64], F32, tag="E")
        nc.vector.tensor_copy(E[:], pk[:])

        def e4(base):  # [128, 8, 4] view of E half
            return E[:, base:base + 32].rearrange("p (ic f) -> p ic f", f=4)

        wm = e4(0)[:, :, 0:3]    # (W1@pos)^T packed
        wl = e4(32)[:, :, 0:3]   # (W2@pos)^T packed

        # replicate R (col 3 of each 4-group) across the 3 channels
        r3m = small.tile([128, 24], F32, tag="r3m")
        r3l = small.tile([128, 24], F32, tag="r3l")
        for c in range(3):
            nc.vector.tensor_copy(
                r3m[:].rearrange("p (ic c) -> p ic c", c=3)[:, :, c:c + 1],
                e4(0)[:, :, 3:4])
            nc.vector.tensor_copy(
                r3l[:].rearrange("p (ic c) -> p ic c", c=3)[:, :, c:c + 1],
                e4(32)[:, :, 3:4])

        def t24(tag):
            return small.tile([128, 24], F32, tag=tag, name=tag)

        mean = t24("mean")
        mean_v = mean[:].rearrange("p (ic c) -> p ic c", c=3)
        nc.vector.tensor_mul(mean[:], pos_pk[:], r3m[:])
        nc.vector.tensor_sub(mean_v, mean_v, wm)

        lstd = t24("lstd")
        lstd_v = lstd[:].rearrange("p (ic c) -> p ic c", c=3)
        nc.vector.tensor_mul(lstd[:], pos_pk[:], r3l[:])
        nc.vector.tensor_sub(lstd_v, lstd_v, wl)
        nc.vector.tensor_scalar(lstd[:], lstd[:], LOG_STD_MIN, LOG_STD_MAX,
                                ALU.max, ALU.min)
        # with accum_out, op1 is the reduce op (add): sums clipped lstd
        s_lstd = small.tile([128, 1], F32, tag="s_lstd")
        lsum = t24("lsum")
        nc.vector.tensor_scalar(lsum[:], lstd[:], 0.0, None,
                                ALU.add, ALU.add, accum_out=s_lstd[:])

        std = t24("std")
        nc.scalar.activation(std[:], lstd[:], AF.Exp)
        pre = t24("pre")
        nc.vector.tensor_mul(pre[:], std[:], eps_pk[:])
        nc.vector.tensor_add(pre[:], mean[:], pre[:])

        # |pre| and u = exp(-2|pre|) (shared by tanh and log-det)
        ap_t = t24("ap_t")
        s_ap = small.tile([128, 1], F32, tag="s_ap")
        nc.scalar.activation(ap_t[:], pre[:], AF.Abs)
        apsum = t24("apsum")
        nc.vector.tensor_scalar(apsum[:], ap_t[:], 0.0, None,
                                ALU.add, ALU.add, accum_out=s_ap[:])
        u_t = t24("u_t")
        nc.scalar.activation(u_t[:], ap_t[:], AF.Exp, scale=-2.0)

        # actions = sign(pre) * (1-u)/(1+u) * ACTION_SCALE
        sgn = t24("sgn")
        nc.scalar.activation(sgn[:], pre[:], AF.Sign)
        den = t24("den")
        nc.vector.tensor_scalar_add(den[:], u_t[:], 1.0)
        rec = t24("rec")
        nc.vector.reciprocal(rec[:], den[:])
        num = t24("num")
        nc.vector.tensor_scalar(num[:], u_t[:], -1.0, 1.0, ALU.mult, ALU.add)
        acts = t24("acts")
        nc.vector.tensor_mul(acts[:], num[:], rec[:])
        nc.vector.tensor_mul(acts[:], acts[:], sgn[:])
        nc.vector.tensor_scalar_mul(acts[:], acts[:], ACTION_SCALE)
        nc.sync.dma_start(act_d[:, :], acts[:])

        # log_prob pieces
        delta = t24("delta")
        nc.vector.tensor_sub(delta[:], pre[:], mean[:])
        istd = t24("istd")
        nc.scalar.activation(istd[:], lstd[:], AF.Exp, scale=-1.0)
        td = t24("td")
        nc.vector.tensor_mul(td[:], delta[:], istd[:])
        sqq = t24("sqq")
        s_sq = small.tile([128, 1], F32, tag="s_sq")
        nc.vector.scalar_tensor_tensor(sqq[:], td[:], 0.0, td[:],
                                       ALU.add, ALU.mult, accum_out=s_sq[:])

        v_t = t24("v_t")
        s_v = small.tile([128, 1], F32, tag="s_v")
        nc.scalar.activation(v_t[:], u_t[:], AF.Ln, bias=1.0)
        vsum = t24("vsum")
        nc.vector.tensor_scalar(vsum[:], v_t[:], 0.0, None,
                                ALU.add, ALU.add, accum_out=s_v[:])

        # L = -0.5*s_sq - s_lstd + 2*s_ap + 2*s_v - 24*(C_NLP + LOG2x2)
        L = small.tile([128, 1], F32, tag="L")
        nc.vector.tensor_scalar_mul(L[:], s_sq[:], -0.5)
        nc.vector.scalar_tensor_tensor(L[:], s_ap[:], 2.0, L[:],
                                       ALU.mult, ALU.add)
        nc.vector.scalar_tensor_tensor(L[:], s_v[:], 2.0, L[:],
                                       ALU.mult, ALU.add)
        nc.vector.scalar_tensor_tensor(L[:], s_lstd[:], -1.0, L[:],
                                       ALU.mult, ALU.add)
        nc.vector.tensor_scalar_add(L[:], L[:], -24.0 * (C_NLP + LOG2x2))

        lp_ps = mmp.tile([1, 1], F32, tag="mm")
        nc.tensor.matmul(lp_ps[:], onescol[:], L[:], start=True, stop=True)
        lp_sb = small.tile([1, 1], F32, tag="lp_sb")
        nc.vector.tensor_scalar_add(lp_sb[:], lp_ps[:], LP_BIAS)
        nc.sync.dma_start(lp_d[:, :], lp_sb[:])

    nc.finalize()
    return nc


_NC_CACHE = None


def _get_nc():
    global _NC_CACHE
    if _NC_CACHE is None:
        _NC_CACHE = build_nc()
    return _NC_CACHE


def make_in_maps(kv, positions, eps):
    bf16 = ml_dtypes.bfloat16
    eyef = np.eye(128, dtype=np.float32)
    eyeba = (np.eye(128) * 1e7).astype(bf16)
    eyebb = (np.eye(128) * -5e6).astype(bf16)
    onescol = np.ones((128, 1), np.float32)
    ones3b = np.ones((3, N), bf16)
    maps = []
    for i in range(B):
        pos = np.ascontiguousarray(positions[i], np.float32)
        epsb = np.ascontiguousarray(eps[i], np.float32)
        h = pos.astype(bf16)
        l = (pos - h.astype(np.float32)).astype(bf16)
        ph = (h.astype(np.float32) + l.astype(np.float32)).astype(np.float32)
        sq = (ph * ph).astype(np.float32)
        r = (sq[:, 0] + sq[:, 1]).astype(np.float32)
        r = (r + sq[:, 2]).astype(np.float32)
        x = (-0.5 * r).astype(np.float32)
        ra = x.astype(bf16)
        x1 = (x - ra.astype(np.float32)).astype(np.float32)
        rb = x1.astype(bf16)
        x2 = (x1 - rb.astype(np.float32)).astype(np.float32)
        rc = x2.astype(bf16)
        hl = np.ascontiguousarray(np.concatenate([h.T, l.T], 0))
        abc = np.ascontiguousarray(np.stack([ra, rb, rc], 0))
        pos_pk = np.ascontiguousarray(
            pos.reshape(8, 128, 3).transpose(1, 0, 2).reshape(128, 24))
        eps_pk = np.ascontiguousarray(
            epsb.reshape(8, 128, 3).transpose(1, 0, 2).reshape(128, 24))
        maps.append({
            "kvt": np.ascontiguousarray(kv[i].T),
            "pos": pos,
            "eps": epsb,
            "eyef": eyef,
            "eyeba": eyeba,
            "eyebb": eyebb,
            "onescol": onescol,
            "ones3b": ones3b,
            "hlrows": hl,
            "abcrows": abc,
            "pospk": pos_pk,
            "epspk": eps_pk,
        })
    return maps


def kernel(kv, positions, eps):
    kv = np.asarray(kv, np.float32)
    positions = np.asarray(positions, np.float32)
    eps = np.asarray(eps, np.float32)
    nc = _get_nc()
    maps = make_in_maps(kv, positions, eps)
    res = run_bass_kernel_spmd(nc, maps, core_ids=list(range(B)))
    outs = res.results
    actions = np.stack([
        outs[i]["actions"].reshape(128, 8, 3).transpose(1, 0, 2)
        .reshape(N, 3) for i in range(B)])
    log_prob = np.stack([outs[i]["logprob"].reshape(1) for i in range(B)])
    return actions.astype(np.float32), log_prob.astype(np.float32)
